# revision 34
# baseline (speedup 1.0000x reference)
"""Trainium2 Bass kernel for nn_BeliefDecoder (LSTM decoder with categorical
sampling), data-parallel over 8 NeuronCores.

Contract: kernel(**inputs) takes FULL unsharded inputs (as produced by
setup_inputs()) and returns the FULL output tuple
(logits (B, 15, 26) f32, samples (B, 15) int32).

Strategy
--------
- Pure data parallel: batch 65536 -> 8 cores x 8192 rows; weights replicated.
- Sampling must be bit-identical to jax.random.categorical: Gumbel noise is
  precomputed on host (CPU jax, threefry -> bit-exact) and shipped to the
  device. jax.random.categorical(k, logits) == argmax(gumbel(k, shape) +
  logits).
- On device, everything runs in a transposed layout [feature/vocab partitions,
  batch free]:
    h0 = tanh(ctx @ Wp + bp)                      (PE + ACT)
    per step: gates = Wh.h + EW'[tok] + xc + b    (PE accumulate + DVE add)
              i,f,g,o nonlinearities              (ACT)
              c,h update                          (DVE)
              logits = Wo.h                       (PE)
              z = logits + G[t]                   (DVE)
              m = max over vocab partitions       (GPSIMD partition_all_reduce)
              onehot = (z == m)                   (DVE is_equal)
  where xc = ctx @ Wi[:256] is precomputed once per batch-tile (context is
  time-invariant) and EW' = embed_table @ Wi[256:] + bh folds the embedding
  lookup into a tiny 26-row matmul against the onehot (the sampled token feeds
  back without ever materialising embeddings).
- The samples themselves are recovered on the host from the logits the kernel
  already outputs: argmax(logits + G) in f32 is bit-identical to the device's
  (z == m) selection.
- Matmuls run in true fp32 (4-pass H/L) so the sampled trajectory tracks the
  f32 reference closely enough that argmax flips are rare.
"""

import os
import numpy as np

H = 256          # hidden
T = 15           # decode steps (num_components)
V = 26           # vocab
E = 64           # embed dim
B = 65536        # batch
NCORES = 8
BT = 256         # batch tile (moving free dim per matmul)
BLOC = B // NCORES
NTILES = BLOC // BT  # 32

_CACHE = {}
LAST_RESULTS = None


def build_nc(n_tiles, use_f32r=False, ileave=2, prod_gp=False,
             pg_bufs=3, kbufs=3, pack_ew=False):
    """Build the Bass/Tile program for one core handling n_tiles*BT rows."""
    import concourse.bass as bass
    import concourse.tile as tile
    import concourse.mybir as mybir
    from concourse import bacc

    f32 = mybir.dt.float32
    wdt = mybir.dt.float32r if use_f32r else f32
    Sig = mybir.ActivationFunctionType.Sigmoid
    Tanh = mybir.ActivationFunctionType.Tanh
    ADD = mybir.AluOpType.add
    ISEQ = mybir.AluOpType.is_equal
    bloc = n_tiles * BT

    nc = bacc.Bacc("TRN2", target_bir_lowering=False, debug=False)

    ctxT = nc.dram_tensor("ctxT", (H, bloc), wdt, kind="ExternalInput")
    gumb = nc.dram_tensor("gumb", (n_tiles, 128, 2, T, V), f32, kind="ExternalInput")
    wp_d = nc.dram_tensor("wp", (H, H), wdt, kind="ExternalInput")
    wi_d = nc.dram_tensor("wi1", (H, 4 * H), wdt, kind="ExternalInput")
    wh_d = nc.dram_tensor("wh", (H, 4 * H), wdt, kind="ExternalInput")
    wo_d = nc.dram_tensor("wo", (H, V), wdt, kind="ExternalInput")
    ew_d = nc.dram_tensor("ew", (V, 4 * H), wdt, kind="ExternalInput")
    cst_d = nc.dram_tensor("consts", (128, 10), f32, kind="ExternalInput")
    id_d = nc.dram_tensor("ident", (128, 128), f32, kind="ExternalInput")
    louts = nc.dram_tensor(
        "louts", (n_tiles, 128, 2, T, V), f32, kind="ExternalOutput"
    )

    def mm(out, lhsT, rhs, family, **kw):
        nc.tensor.matmul(out, lhsT, rhs, **kw)

    # (i,g) then (f,o): c-update needs i,g early; h-update needs f,o late.
    HALF_CHUNKS = [[0, 1, 4, 5], [2, 3, 6, 7]]

    with tile.TileContext(nc) as tc:
        with (
            tc.tile_pool(name="weights", bufs=1) as wpool,
            tc.tile_pool(name="state", bufs=ileave + 1) as spool,
            tc.tile_pool(name="work", bufs=kbufs) as kpool,
            tc.tile_pool(name="oh", bufs=2 * ileave) as ohpool,
            tc.tile_pool(name="stage", bufs=ileave + 1) as stpool,
            tc.tile_pool(name="pgates", bufs=pg_bufs, space="PSUM") as pg_pool,
            tc.tile_pool(name="plog", bufs=1, space="PSUM") as pl_pool,
            tc.tile_pool(name="pbc", bufs=1, space="PSUM") as pbc_pool,
        ):
            # ---- load weights (once) ----
            wp_sb = wpool.tile([128, 2, 2, 128], wdt, tag="wp")
            nc.sync.dma_start(
                wp_sb[:], wp_d.rearrange("(k p) (m c) -> p k m c", p=128, c=128)
            )
            wi_sb = wpool.tile([128, 2, 8, 128], wdt, tag="wi")
            nc.sync.dma_start(
                wi_sb[:], wi_d.rearrange("(k p) (m c) -> p k m c", p=128, c=128)
            )
            wh_sb = wpool.tile([128, 2, 8, 128], wdt, tag="wh")
            nc.sync.dma_start(
                wh_sb[:], wh_d.rearrange("(k p) (m c) -> p k m c", p=128, c=128)
            )
            wo_sb = wpool.tile([128, 2, V], wdt, tag="wo")
            nc.sync.dma_start(wo_sb[:], wo_d.rearrange("(k p) v -> p k v", p=128))
            if pack_ew:
                # EW replicated at partition offsets 0/32/64/96 for 4-way
                # row-group packed matmuls
                ew_sb = wpool.tile([128, 8, 128], wdt, tag="ew")
                for r in range(4):
                    nc.sync.dma_start(
                        ew_sb[32 * r : 32 * r + V, :, :],
                        ew_d.rearrange("v (m c) -> v m c", c=128),
                    )
            else:
                ew_sb = wpool.tile([V, 8, 128], wdt, tag="ew")
                nc.sync.dma_start(ew_sb[:], ew_d.rearrange("v (m c) -> v m c", c=128))
            cst = wpool.tile([128, 10], f32, tag="cst")
            nc.sync.dma_start(cst[:], cst_d[:, :])
            id_sb = wpool.tile([128, 128], f32, tag="ident")
            nc.sync.dma_start(id_sb[:], id_d[:, :])

            state = {}  # per-tile persistent tiles
            prev_oh = {}

            def setup(j):
                ctx_sb = spool.tile([128, 2, BT], wdt, tag="ctx")
                nc.sync.dma_start(
                    ctx_sb[:],
                    ctxT.rearrange("(k p) b -> p k b", p=128)[
                        :, :, j * BT : (j + 1) * BT
                    ],
                )
                g_sb = stpool.tile([128, 2, T, V], f32, tag="gum")
                nc.sync.dma_start(g_sb[:], gumb[j])

                # h0 = tanh(ctx @ Wp + bp)
                hT = spool.tile([128, 2, BT], wdt, tag="h")
                cT = spool.tile([128, 2, BT], f32, tag="c")
                nc.vector.memset(cT[:], 0.0)
                p_wp = pg_pool.tile([128, 4, BT], f32, tag="pg")
                for m in range(2):
                    for k in range(2):
                        mm(
                            p_wp[:, m, :], wp_sb[:, k, m, :], ctx_sb[:, k, :],
                            "wp", start=(k == 0), stop=(k == 1),
                        )
                for m in range(2):
                    nc.scalar.activation(
                        hT[:, m, :], p_wp[:, m, :], Tanh, bias=cst[:, 8 + m : 9 + m]
                    )

                lstage = stpool.tile([128, 2, T, V], f32, tag="lst")
                state[j] = (hT, cT, ctx_sb, g_sb, lstage)

            def step(j, t):
                hT, cT, ctx_sb, g_sb, lstage = state[j]
                # gates, two PSUM halves of 2 banks each:
                #   gates = Wh.h + Wi[:256].ctx (+ EW'[tok_prev]); ACT reads PSUM
                acts = kpool.tile([128, 2, 4, BT], f32, tag="acts")
                t1 = kpool.tile([128, 2, BT], f32, tag="t1")
                t2 = kpool.tile([128, 2, BT], f32, tag="t2")
                for half in range(2):
                    p_h = pg_pool.tile([128, 4, BT], f32, tag="pg")
                    for pos, m in enumerate(HALF_CHUNKS[half]):
                        for k in range(2):
                            mm(
                                p_h[:, pos, :], wh_sb[:, k, m, :], hT[:, k, :],
                                "wh", start=(k == 0), stop=False,
                            )
                        for k in range(2):
                            mm(
                                p_h[:, pos, :], wi_sb[:, k, m, :], ctx_sb[:, k, :],
                                "xc", start=False, stop=(t == 0 and k == 1),
                            )
                        if t > 0:
                            if pack_ew:
                                r = pos  # row group = position within half
                                nc.tensor.matmul(
                                    p_h[:, pos, :],
                                    ew_sb[32 * r : 32 * r + V, m, :],
                                    prev_oh[j][32 * r : 32 * r + V, :],
                                    start=False, stop=True,
                                    tile_position=(32 * r, 0),
                                )
                            else:
                                mm(
                                    p_h[:, pos, :], ew_sb[:, m, :], prev_oh[j][:],
                                    "ew", start=False, stop=True,
                                )
                    # nonlinearities straight from PSUM; bias bh(+sE) only at
                    # t=0 (EW' carries bh for t>0)
                    if t == 0:
                        for pos, m in enumerate(HALF_CHUNKS[half]):
                            func = Tanh if m in (4, 5) else Sig
                            nc.scalar.activation(
                                acts[:, half, pos, :], p_h[:, pos, :], func,
                                bias=cst[:, m : m + 1],
                            )
                    else:
                        nc.scalar.activation(
                            acts[:, half, 0:2, :], p_h[:, 0:2, :], Sig
                        )
                        nc.scalar.activation(
                            acts[:, half, 2:4, :], p_h[:, 2:4, :],
                            Tanh if half == 0 else Sig,
                        )
                    mule = nc.gpsimd if prod_gp else nc.vector
                    if half == 0:
                        # t2 = sig(i)*tanh(g)
                        mule.tensor_mul(
                            t2[:], acts[:, 0, 0:2, :], acts[:, 0, 2:4, :]
                        )
                    else:
                        # c = sig(f)*c + t2 ; h = sig(o)*tanh(c)
                        mule.tensor_mul(t1[:], acts[:, 1, 0:2, :], cT[:])
                        nc.vector.tensor_add(cT[:], t1[:], t2[:])
                        tct = kpool.tile([128, 2, BT], f32, tag="tct")
                        nc.scalar.activation(tct[:], cT[:], Tanh)
                        mule.tensor_mul(hT[:], acts[:, 1, 2:4, :], tct[:])

                # logits in BATCH-major: out[batch 128, 26] via hT-as-stationary
                p_lb = pl_pool.tile([128, 2, V], f32, tag="plb")
                for c in range(2):
                    for k in range(2):
                        mm(
                            p_lb[:, c, :],
                            hT[:, k, c * 128 : (c + 1) * 128],
                            wo_sb[:, k, :],
                            "wo", start=(k == 0), stop=(k == 1),
                        )
                nc.scalar.copy(lstage[:, :, t, :], p_lb[:])

                if t < T - 1:
                    # z = logits + gumbel'; argmax along the free (vocab) dim;
                    # onehot back to vocab-major via PE transpose for feedback.
                    zt = kpool.tile([128, 2, V], f32, tag="zt")
                    nc.vector.tensor_add(zt[:], p_lb[:], g_sb[:, :, t, :])
                    mt = kpool.tile([128, 2], f32, tag="mt")
                    nc.vector.tensor_reduce(
                        mt[:], zt[:], axis=mybir.AxisListType.X,
                        op=mybir.AluOpType.max,
                    )
                    eqt = kpool.tile([128, 2, V], f32, tag="eqt")
                    for c in range(2):
                        nc.vector.tensor_scalar(
                            eqt[:, c, :], zt[:, c, :], mt[:, c : c + 1], None,
                            op0=ISEQ,
                        )
                    p_oh = pbc_pool.tile([V, 2, 128], f32, tag="poh")
                    for c in range(2):
                        nc.tensor.transpose(p_oh[:, c, :], eqt[:, c, :], id_sb[:])
                    if pack_ew:
                        oh = ohpool.tile([128, BT], wdt, tag="oh")
                        nc.scalar.copy(oh[0:V, :], p_oh[:])
                        for r in range(1, 4):
                            nc.vector.tensor_copy(
                                oh[32 * r : 32 * r + V, :], oh[0:V, :]
                            )
                    else:
                        oh = ohpool.tile([V, BT], wdt, tag="oh")
                        nc.scalar.copy(oh[:], p_oh[:])
                    prev_oh[j] = oh

            def finish(j):
                lstage = state[j][4]
                nc.sync.dma_start(louts[j], lstage[:])
                del state[j]
                prev_oh.pop(j, None)

            for jj in range(0, n_tiles, ileave):
                grp = list(range(jj, min(jj + ileave, n_tiles)))
                for j in grp:
                    setup(j)
                for t in range(T):
                    for j in grp:
                        step(j, t)
                for j in grp:
                    finish(j)

    nc.compile()
    return nc


def _prep_host(context, embed_table, start_embed, Wp, bp, Wi, Wh, bh, Wo, bo, seed):
    """Host-side preprocessing: gumbel noise, weight repacks, per-core shards."""
    import jax
    import jax.numpy as jnp

    cpu = jax.devices("cpu")[0]
    with jax.default_device(cpu):
        keys = jax.random.split(jax.random.key(int(seed)), T)
        # per-key gumbel calls: bit-exact with jax.random.categorical's
        # internal noise (NOTE: vmap over keys yields different bits!)
        gfn = jax.jit(lambda k: jax.random.gumbel(k, (B, V), jnp.float32))
        G = np.stack([np.asarray(gfn(keys[t])) for t in range(T)])  # (T, B, V)
    bo = np.asarray(bo, np.float32)
    Gp = (G + bo[None, None, :]).astype(np.float32)  # z = logits + (gumbel + bo)

    Wi = np.asarray(Wi)
    bh = np.asarray(bh)
    ew = (
        np.asarray(embed_table).astype(np.float64) @ Wi[H:].astype(np.float64)
        + bh.astype(np.float64)
    ).astype(np.float32)  # (V, 4H)
    bias0 = (
        np.asarray(start_embed).astype(np.float64) @ Wi[H:].astype(np.float64)
        + bh.astype(np.float64)
    ).astype(np.float32)  # (4H,)
    consts = np.zeros((128, 10), np.float32)
    for ch in range(8):
        consts[:, ch] = bias0[ch * 128 : (ch + 1) * 128]
    bp = np.asarray(bp, np.float32)
    consts[:, 8] = bp[:128]
    consts[:, 9] = bp[128:]

    context = np.asarray(context, np.float32)
    in_maps = []
    for c in range(NCORES):
        sl = slice(c * BLOC, (c + 1) * BLOC)
        ctxT = np.ascontiguousarray(context[sl].T)  # (H, BLOC)
        # (T, BLOC, V) -> (NTILES, 128, 2, T, V); b = j*256 + c*128 + p
        gc = np.ascontiguousarray(
            Gp[:, sl, :].reshape(T, NTILES, 2, 128, V).transpose(1, 3, 2, 0, 4)
        )
        in_maps.append(
            dict(
                ctxT=ctxT,
                gumb=gc,
                wp=np.asarray(Wp, np.float32),
                wi1=Wi[:H].astype(np.float32),
                wh=np.asarray(Wh, np.float32),
                wo=np.asarray(Wo, np.float32),
                ew=ew,
                consts=consts,
                ident=np.eye(128, dtype=np.float32),
            )
        )
    return in_maps, Gp, bo


def _get_runner():
    """Build (once) a cached sharded-PJRT executable for the kernel, modeled on
    concourse.bass2jax.run_bass_via_pjrt's multi-core path, plus a bench()
    that times steady-state execution with device-resident inputs."""
    if "runner" in _CACHE:
        return _CACHE["runner"]
    import jax
    import jax.numpy as jnp
    from jax.sharding import Mesh, PartitionSpec, NamedSharding
    from jax.experimental.shard_map import shard_map
    import concourse.mybir as mybir
    from concourse import bass2jax

    use_f32r = os.environ.get("TRN_F32R", "1") == "1"
    nc = _CACHE.get(("nc", NTILES, use_f32r))
    if nc is None:
        nc = _CACHE[("nc", NTILES, use_f32r)] = build_nc(
            NTILES, use_f32r, ileave=4, pack_ew=True
        )
    bass2jax.install_neuronx_cc_hook()

    partition_name = (
        nc.partition_id_tensor.name if nc.partition_id_tensor else None
    )
    in_names, out_names, out_avals, zero_shapes = [], [], [], []
    for alloc in nc.m.functions[0].allocations:
        if not isinstance(alloc, mybir.MemoryLocationSet):
            continue
        name = alloc.memorylocations[0].name
        if alloc.kind == "ExternalInput":
            if name != partition_name:
                in_names.append(name)
        elif alloc.kind == "ExternalOutput":
            shape = tuple(alloc.tensor_shape)
            dtype = mybir.dt.np(alloc.dtype)
            out_names.append(name)
            out_avals.append(jax.core.ShapedArray(shape, dtype))
            zero_shapes.append((shape, dtype))
    n_params = len(in_names)
    n_outs = len(out_names)
    all_names = tuple(
        in_names + out_names + ([partition_name] if partition_name else [])
    )

    def _body(*args):
        operands = list(args)
        if partition_name is not None:
            operands.append(bass2jax.partition_id_tensor())
        outs = bass2jax._bass_exec_p.bind(
            *operands,
            out_avals=tuple(out_avals),
            in_names=all_names,
            out_names=tuple(out_names),
            lowering_input_output_aliases=(),
            sim_require_finite=True,
            sim_require_nnan=True,
            nc=nc,
        )
        return tuple(outs)

    devices = jax.devices()[:NCORES]
    mesh = Mesh(np.asarray(devices), ("core",))
    pspec = PartitionSpec("core")
    sharded = jax.jit(
        shard_map(
            _body,
            mesh=mesh,
            in_specs=(pspec,) * (n_params + n_outs),
            out_specs=(pspec,) * n_outs,
            check_rep=False,
        ),
        donate_argnums=tuple(range(n_params, n_params + n_outs)),
        keep_unused=True,
    )
    zeros_fn = jax.jit(
        lambda: tuple(
            jnp.zeros((NCORES * s[0], *s[1:]), d) for s, d in zero_shapes
        ),
        out_shardings=tuple(NamedSharding(mesh, pspec) for _ in zero_shapes),
    )

    def run(in_maps):
        concat_in = [
            np.concatenate([m[name] for m in in_maps], axis=0) for name in in_names
        ]
        dev_in = [jax.device_put(a, NamedSharding(mesh, pspec)) for a in concat_in]
        out = sharded(*dev_in, *zeros_fn())
        results = []
        for c in range(NCORES):
            results.append(
                {
                    name: np.asarray(out[i]).reshape(NCORES, *out_avals[i].shape)[c]
                    for i, name in enumerate(out_names)
                }
            )
        return results, dev_in

    def bench(dev_in, iters=3):
        import time

        times = []
        for _ in range(iters):
            zs = jax.block_until_ready(zeros_fn())
            t0 = time.perf_counter()
            out = sharded(*dev_in, *zs)
            jax.block_until_ready(out)
            times.append(time.perf_counter() - t0)
        return times

    _CACHE["runner"] = (run, bench)
    return _CACHE["runner"]


def bench_exec(iters=3):
    """Steady-state wall time per execution (device-resident inputs)."""
    run, bench = _get_runner()
    dev_in = _CACHE.get("last_dev_in")
    if dev_in is None:
        raise RuntimeError("call kernel() first")
    return bench(dev_in, iters)


def kernel(context, embed_table, start_embed, Wp, bp, Wi, Wh, bh, Wo, bo, seed):
    global LAST_RESULTS
    run, _ = _get_runner()
    in_maps, Gp, bo_np = _prep_host(
        context, embed_table, start_embed, Wp, bp, Wi, Wh, bh, Wo, bo, seed
    )
    results, dev_in = run(in_maps)
    _CACHE["last_dev_in"] = dev_in
    res = type("R", (), {"results": results})()
    LAST_RESULTS = res

    logits = np.empty((B, T, V), np.float32)
    samples = np.empty((B, T), np.int32)
    for c in range(NCORES):
        lc = res.results[c]["louts"]  # (NTILES, 128, 2, T, V)
        # device z == lc + gumb bit-exactly -> samples match device feedback
        zc = lc + in_maps[c]["gumb"]
        sc = np.argmax(zc, axis=-1)  # (NTILES, 128, 2, T)
        sl = slice(c * BLOC, (c + 1) * BLOC)
        logits[sl] = lc.transpose(0, 2, 1, 3, 4).reshape(BLOC, T, V)
        samples[sl] = sc.transpose(0, 2, 1, 3).reshape(BLOC, T).astype(np.int32)
    logits += bo_np[None, None, :]
    return logits, samples


# revision 41
# speedup vs baseline: 18.0513x; 18.0513x over previous
"""Trainium2 Bass kernel for nn_BeliefDecoder (LSTM decoder with categorical
sampling), data-parallel over 8 NeuronCores.

Contract: kernel(**inputs) takes FULL unsharded inputs (as produced by
setup_inputs()) and returns the FULL output tuple
(logits (B, 15, 26) f32, samples (B, 15) int32).

Strategy
--------
- Pure data parallel: batch 65536 -> 8 cores x 8192 rows; weights replicated.
- Sampling must be bit-identical to jax.random.categorical: Gumbel noise is
  precomputed on host (CPU jax, threefry -> bit-exact) and shipped to the
  device. jax.random.categorical(k, logits) == argmax(gumbel(k, shape) +
  logits).
- On device, everything runs in a transposed layout [feature/vocab partitions,
  batch free]:
    h0 = tanh(ctx @ Wp + bp)                      (PE + ACT)
    per step: gates = Wh.h + EW'[tok] + xc + b    (PE accumulate + DVE add)
              i,f,g,o nonlinearities              (ACT)
              c,h update                          (DVE)
              logits = Wo.h                       (PE)
              z = logits + G[t]                   (DVE)
              m = max over vocab partitions       (GPSIMD partition_all_reduce)
              onehot = (z == m)                   (DVE is_equal)
  where xc = ctx @ Wi[:256] is precomputed once per batch-tile (context is
  time-invariant) and EW' = embed_table @ Wi[256:] + bh folds the embedding
  lookup into a tiny 26-row matmul against the onehot (the sampled token feeds
  back without ever materialising embeddings).
- The samples themselves are recovered on the host from the logits the kernel
  already outputs: argmax(logits + G) in f32 is bit-identical to the device's
  (z == m) selection.
- Matmuls run in true fp32 (4-pass H/L) so the sampled trajectory tracks the
  f32 reference closely enough that argmax flips are rare.
"""

import os
import numpy as np

H = 256          # hidden
T = 15           # decode steps (num_components)
V = 26           # vocab
E = 64           # embed dim
B = 65536        # batch
NCORES = 8
BT = 256         # batch tile (moving free dim per matmul)
BLOC = B // NCORES
NTILES = BLOC // BT  # 32

_CACHE = {}
LAST_RESULTS = None


def build_nc(n_tiles, use_f32r=False, ileave=2, prod_gp=False,
             pg_bufs=3, kbufs=3, pack_ew=False, reps=1, off_gp=False,
             bt=None):
    """Build the Bass/Tile program for one core handling n_tiles*BT rows."""
    import concourse.bass as bass
    import concourse.tile as tile
    import concourse.mybir as mybir
    from concourse import bacc

    f32 = mybir.dt.float32
    wdt = mybir.dt.float32r if use_f32r else f32
    Sig = mybir.ActivationFunctionType.Sigmoid
    Tanh = mybir.ActivationFunctionType.Tanh
    ADD = mybir.AluOpType.add
    ISEQ = mybir.AluOpType.is_equal
    bt = BT if bt is None else bt
    bloc = n_tiles * bt
    nc2 = bt // 128  # batch sub-chunks of 128 for batch-major logits/sampling
    gpq = 512 // bt  # gates per PSUM group: BT=256 -> 2 (i,g)/(f,o); BT=512 -> 1

    nc = bacc.Bacc("TRN2", target_bir_lowering=False, debug=False)

    ctxT = nc.dram_tensor("ctxT", (H, bloc), wdt, kind="ExternalInput")
    gumb = nc.dram_tensor("gumb", (n_tiles, 128, nc2, T, V), f32, kind="ExternalInput")
    wp_d = nc.dram_tensor("wp", (H, H), wdt, kind="ExternalInput")
    wi_d = nc.dram_tensor("wi1", (H, 4 * H), wdt, kind="ExternalInput")
    wh_d = nc.dram_tensor("wh", (H, 4 * H), wdt, kind="ExternalInput")
    wo_d = nc.dram_tensor("wo", (H, V), wdt, kind="ExternalInput")
    ew_d = nc.dram_tensor("ew", (V, 4 * H), wdt, kind="ExternalInput")
    cst_d = nc.dram_tensor("consts", (128, 10), f32, kind="ExternalInput")
    id_d = nc.dram_tensor("ident", (128, 128), f32, kind="ExternalInput")
    louts = nc.dram_tensor(
        "louts", (n_tiles, 128, nc2, T, V), f32, kind="ExternalOutput"
    )

    def mm(out, lhsT, rhs, family, **kw):
        nc.tensor.matmul(out, lhsT, rhs, **kw)

    # gate processing groups: c-update needs i,g early; h-update needs f,o late.
    # chunk c of 4H belongs to gate c//2 (order i,f,g,o).
    if gpq == 2:
        GROUPS = [[0, 2], [1, 3]]      # (i,g) then (f,o)
    else:
        GROUPS = [[0], [2], [1], [3]]  # i, g, f, o

    with tile.TileContext(nc) as tc:
        with (
            tc.tile_pool(name="weights", bufs=1) as wpool,
            tc.tile_pool(name="state", bufs=ileave + 1) as spool,
            tc.tile_pool(name="work", bufs=kbufs) as kpool,
            tc.tile_pool(name="oh", bufs=2 * ileave) as ohpool,
            tc.tile_pool(name="stage", bufs=ileave + 1) as stpool,
            tc.tile_pool(name="pgates", bufs=pg_bufs, space="PSUM") as pg_pool,
            tc.tile_pool(name="plog", bufs=1, space="PSUM") as pl_pool,
            tc.tile_pool(name="pbc", bufs=1, space="PSUM") as pbc_pool,
        ):
            # ---- load weights (once) ----
            wp_sb = wpool.tile([128, 2, 2, 128], wdt, tag="wp")
            nc.sync.dma_start(
                wp_sb[:], wp_d.rearrange("(k p) (m c) -> p k m c", p=128, c=128)
            )
            wi_sb = wpool.tile([128, 2, 8, 128], wdt, tag="wi")
            nc.sync.dma_start(
                wi_sb[:], wi_d.rearrange("(k p) (m c) -> p k m c", p=128, c=128)
            )
            wh_sb = wpool.tile([128, 2, 8, 128], wdt, tag="wh")
            nc.sync.dma_start(
                wh_sb[:], wh_d.rearrange("(k p) (m c) -> p k m c", p=128, c=128)
            )
            wo_sb = wpool.tile([128, 2, V], wdt, tag="wo")
            nc.sync.dma_start(wo_sb[:], wo_d.rearrange("(k p) v -> p k v", p=128))
            if pack_ew:
                # EW replicated at partition offsets 0/32/64/96 for 4-way
                # row-group packed matmuls
                ew_sb = wpool.tile([128, 8, 128], wdt, tag="ew")
                for r in range(4):
                    nc.sync.dma_start(
                        ew_sb[32 * r : 32 * r + V, :, :],
                        ew_d.rearrange("v (m c) -> v m c", c=128),
                    )
            else:
                ew_sb = wpool.tile([V, 8, 128], wdt, tag="ew")
                nc.sync.dma_start(ew_sb[:], ew_d.rearrange("v (m c) -> v m c", c=128))
            cst = wpool.tile([128, 10], f32, tag="cst")
            nc.sync.dma_start(cst[:], cst_d[:, :])
            id_sb = wpool.tile([128, 128], f32, tag="ident")
            nc.sync.dma_start(id_sb[:], id_d[:, :])

            state = {}  # per-tile persistent tiles
            kpool_last_tct = [None]
            prev_oh = {}

            def setup(j):
                ctx_sb = spool.tile([128, 2, bt], wdt, tag="ctx")
                nc.sync.dma_start(
                    ctx_sb[:],
                    ctxT.rearrange("(k p) b -> p k b", p=128)[
                        :, :, j * BT : (j + 1) * BT
                    ],
                )
                g_sb = stpool.tile([128, nc2, T, V], f32, tag="gum")
                nc.sync.dma_start(g_sb[:], gumb[j])

                # h0 = tanh(ctx @ Wp + bp)
                hT = spool.tile([128, 2, bt], wdt, tag="h")
                cT = spool.tile([128, 2, bt], f32, tag="c")
                nc.vector.memset(cT[:], 0.0)
                p_wp = pg_pool.tile([128, 2 * gpq, bt], f32, tag="pg")
                for m in range(2):
                    for k in range(2):
                        mm(
                            p_wp[:, m, :], wp_sb[:, k, m, :], ctx_sb[:, k, :],
                            "wp", start=(k == 0), stop=(k == 1),
                        )
                for m in range(2):
                    nc.scalar.activation(
                        hT[:, m, :], p_wp[:, m, :], Tanh, bias=cst[:, 8 + m : 9 + m]
                    )

                lstage = stpool.tile([128, nc2, T, V], f32, tag="lst")
                state[j] = (hT, cT, ctx_sb, g_sb, lstage)

            def step(j, t):
                hT, cT, ctx_sb, g_sb, lstage = state[j]
                # gates, PSUM groups of 2 banks each:
                #   gates = Wh.h + Wi[:256].ctx (+ EW'[tok_prev]); ACT reads PSUM
                acts = kpool.tile([128, 4, 2, bt], f32, tag="acts")
                t1 = kpool.tile([128, 2, bt], f32, tag="t1")
                t2 = kpool.tile([128, 2, bt], f32, tag="t2")
                nrg = 2 * gpq  # row groups used for packed EW matmuls
                for gi, gates in enumerate(GROUPS):
                    p_h = pg_pool.tile([128, 2 * gpq, bt], f32, tag="pg")
                    for pg_i, gate in enumerate(gates):
                        for c2 in range(2):
                            pos = pg_i * 2 + c2
                            m = gate * 2 + c2
                            for k in range(2):
                                mm(
                                    p_h[:, pos, :], wh_sb[:, k, m, :], hT[:, k, :],
                                    "wh", start=(k == 0), stop=False,
                                )
                            for k in range(2):
                                mm(
                                    p_h[:, pos, :], wi_sb[:, k, m, :],
                                    ctx_sb[:, k, :],
                                    "xc", start=False, stop=(t == 0 and k == 1),
                                )
                            if t > 0:
                                if pack_ew:
                                    r = pos  # row group = position within group
                                    nc.tensor.matmul(
                                        p_h[:, pos, :],
                                        ew_sb[32 * r : 32 * r + V, m, :],
                                        prev_oh[j][32 * r : 32 * r + V, :],
                                        start=False, stop=True,
                                        tile_position=(32 * r, 0),
                                    )
                                else:
                                    mm(
                                        p_h[:, pos, :], ew_sb[:, m, :],
                                        prev_oh[j][0:V, :],
                                        "ew", start=False, stop=True,
                                    )
                    # nonlinearities straight from PSUM; bias bh(+sE) only at
                    # t=0 (EW' carries bh for t>0)
                    for pg_i, gate in enumerate(gates):
                        func = Tanh if gate == 2 else Sig
                        if t == 0:
                            for c2 in range(2):
                                m = gate * 2 + c2
                                nc.scalar.activation(
                                    acts[:, gate, c2, :],
                                    p_h[:, pg_i * 2 + c2, :], func,
                                    bias=cst[:, m : m + 1],
                                )
                        else:
                            nc.scalar.activation(
                                acts[:, gate, :, :],
                                p_h[:, pg_i * 2 : pg_i * 2 + 2, :], func,
                            )
                    done = set(g for gg in GROUPS[: gi + 1] for g in gg)
                    just = set(gates)
                    mule = nc.gpsimd if prod_gp else nc.vector
                    if 2 in just and 0 in done:
                        # t2 = sig(i)*tanh(g) (off critical path)
                        (nc.gpsimd if off_gp else mule).tensor_mul(
                            t2[:], acts[:, 0, :, :], acts[:, 2, :, :]
                        )
                    if 1 in just:
                        # c = sig(f)*c + t2
                        mule.tensor_mul(t1[:], acts[:, 1, :, :], cT[:])
                        nc.vector.tensor_add(cT[:], t1[:], t2[:])
                        tct = kpool.tile([128, 2, bt], f32, tag="tct")
                        nc.scalar.activation(tct[:], cT[:], Tanh)
                        kpool_last_tct[0] = tct
                    if 3 in just:
                        # h = sig(o)*tanh(c)
                        tct = kpool_last_tct[0]
                        mule.tensor_mul(hT[:], acts[:, 3, :, :], tct[:])

                # logits in BATCH-major: out[batch 128, 26] via hT-as-stationary
                p_lb = pl_pool.tile([128, nc2, V], f32, tag="plb")
                for c in range(nc2):
                    for k in range(2):
                        mm(
                            p_lb[:, c, :],
                            hT[:, k, c * 128 : (c + 1) * 128],
                            wo_sb[:, k, :],
                            "wo", start=(k == 0), stop=(k == 1),
                        )
                nc.scalar.copy(lstage[:, :, t, :], p_lb[:])

                if t < T - 1:
                    # z = logits + gumbel'; argmax along the free (vocab) dim;
                    # onehot back to vocab-major via PE transpose for feedback.
                    zt = kpool.tile([128, nc2, V], f32, tag="zt")
                    nc.vector.tensor_add(zt[:], p_lb[:], g_sb[:, :, t, :])
                    mt = kpool.tile([128, nc2], f32, tag="mt")
                    nc.vector.tensor_reduce(
                        mt[:], zt[:], axis=mybir.AxisListType.X,
                        op=mybir.AluOpType.max,
                    )
                    eqt = kpool.tile([128, nc2, V], f32, tag="eqt")
                    for c in range(nc2):
                        nc.vector.tensor_scalar(
                            eqt[:, c, :], zt[:, c, :], mt[:, c : c + 1], None,
                            op0=ISEQ,
                        )
                    p_oh = pbc_pool.tile([V, nc2, 128], f32, tag="poh")
                    for c in range(nc2):
                        nc.tensor.transpose(p_oh[:, c, :], eqt[:, c, :], id_sb[:])
                    if pack_ew:
                        oh = ohpool.tile([32 * (nrg - 1) + V, bt], wdt, tag="oh")
                        nc.scalar.copy(oh[0:V, :], p_oh[:])
                        for r in range(1, nrg):
                            (nc.gpsimd if off_gp else nc.vector).tensor_copy(
                                oh[32 * r : 32 * r + V, :], oh[0:V, :]
                            )
                    else:
                        oh = ohpool.tile([V, bt], wdt, tag="oh")
                        nc.scalar.copy(oh[:], p_oh[:])
                    prev_oh[j] = oh

            def finish(j):
                lstage = state[j][4]
                nc.sync.dma_start(louts[j], lstage[:])
                del state[j]
                prev_oh.pop(j, None)

            def whole_pass():
                for jj in range(0, n_tiles, ileave):
                    grp = list(range(jj, min(jj + ileave, n_tiles)))
                    for j in grp:
                        setup(j)
                    for t in range(T):
                        for j in grp:
                            step(j, t)
                    for j in grp:
                        finish(j)

            if reps == 1:
                whole_pass()
            else:
                # benchmark-only: repeat the whole computation on-device so
                # the per-execution time is resolvable above host RPC noise
                with tc.For_i(0, reps, 1):
                    whole_pass()

    nc.compile()
    return nc


def _prep_host(context, embed_table, start_embed, Wp, bp, Wi, Wh, bh, Wo, bo, seed):
    """Host-side preprocessing: gumbel noise, weight repacks, per-core shards."""
    import jax
    import jax.numpy as jnp

    cpu = jax.devices("cpu")[0]
    with jax.default_device(cpu):
        keys = jax.random.split(jax.random.key(int(seed)), T)
        # per-key gumbel calls: bit-exact with jax.random.categorical's
        # internal noise (NOTE: vmap over keys yields different bits!)
        gfn = jax.jit(lambda k: jax.random.gumbel(k, (B, V), jnp.float32))
        G = np.stack([np.asarray(gfn(keys[t])) for t in range(T)])  # (T, B, V)
    bo = np.asarray(bo, np.float32)
    Gp = (G + bo[None, None, :]).astype(np.float32)  # z = logits + (gumbel + bo)

    Wi = np.asarray(Wi)
    bh = np.asarray(bh)
    ew = (
        np.asarray(embed_table).astype(np.float64) @ Wi[H:].astype(np.float64)
        + bh.astype(np.float64)
    ).astype(np.float32)  # (V, 4H)
    bias0 = (
        np.asarray(start_embed).astype(np.float64) @ Wi[H:].astype(np.float64)
        + bh.astype(np.float64)
    ).astype(np.float32)  # (4H,)
    consts = np.zeros((128, 10), np.float32)
    for ch in range(8):
        consts[:, ch] = bias0[ch * 128 : (ch + 1) * 128]
    bp = np.asarray(bp, np.float32)
    consts[:, 8] = bp[:128]
    consts[:, 9] = bp[128:]

    context = np.asarray(context, np.float32)
    in_maps = []
    for c in range(NCORES):
        sl = slice(c * BLOC, (c + 1) * BLOC)
        ctxT = np.ascontiguousarray(context[sl].T)  # (H, BLOC)
        # (T, BLOC, V) -> (NTILES, 128, BT//128, T, V); b = j*BT + c*128 + p
        gc = np.ascontiguousarray(
            Gp[:, sl, :]
            .reshape(T, NTILES, BT // 128, 128, V)
            .transpose(1, 3, 2, 0, 4)
        )
        in_maps.append(
            dict(
                ctxT=ctxT,
                gumb=gc,
                wp=np.asarray(Wp, np.float32),
                wi1=Wi[:H].astype(np.float32),
                wh=np.asarray(Wh, np.float32),
                wo=np.asarray(Wo, np.float32),
                ew=ew,
                consts=consts,
                ident=np.eye(128, dtype=np.float32),
            )
        )
    return in_maps, Gp, bo


def _make_runner(nc):
    """Build a sharded-PJRT executable for a built Bass program, modeled on
    concourse.bass2jax.run_bass_via_pjrt's multi-core path, plus a bench()
    that times steady-state execution with device-resident inputs."""
    import jax
    import jax.numpy as jnp
    from jax.sharding import Mesh, PartitionSpec, NamedSharding
    from jax.experimental.shard_map import shard_map
    import concourse.mybir as mybir
    from concourse import bass2jax

    bass2jax.install_neuronx_cc_hook()

    partition_name = (
        nc.partition_id_tensor.name if nc.partition_id_tensor else None
    )
    in_names, out_names, out_avals, zero_shapes = [], [], [], []
    for alloc in nc.m.functions[0].allocations:
        if not isinstance(alloc, mybir.MemoryLocationSet):
            continue
        name = alloc.memorylocations[0].name
        if alloc.kind == "ExternalInput":
            if name != partition_name:
                in_names.append(name)
        elif alloc.kind == "ExternalOutput":
            shape = tuple(alloc.tensor_shape)
            dtype = mybir.dt.np(alloc.dtype)
            out_names.append(name)
            out_avals.append(jax.core.ShapedArray(shape, dtype))
            zero_shapes.append((shape, dtype))
    n_params = len(in_names)
    n_outs = len(out_names)
    all_names = tuple(
        in_names + out_names + ([partition_name] if partition_name else [])
    )

    def _body(*args):
        operands = list(args)
        if partition_name is not None:
            operands.append(bass2jax.partition_id_tensor())
        outs = bass2jax._bass_exec_p.bind(
            *operands,
            out_avals=tuple(out_avals),
            in_names=all_names,
            out_names=tuple(out_names),
            lowering_input_output_aliases=(),
            sim_require_finite=True,
            sim_require_nnan=True,
            nc=nc,
        )
        return tuple(outs)

    devices = jax.devices()[:NCORES]
    mesh = Mesh(np.asarray(devices), ("core",))
    pspec = PartitionSpec("core")
    sharded = jax.jit(
        shard_map(
            _body,
            mesh=mesh,
            in_specs=(pspec,) * (n_params + n_outs),
            out_specs=(pspec,) * n_outs,
            check_rep=False,
        ),
        donate_argnums=tuple(range(n_params, n_params + n_outs)),
        keep_unused=True,
    )
    zeros_fn = jax.jit(
        lambda: tuple(
            jnp.zeros((NCORES * s[0], *s[1:]), d) for s, d in zero_shapes
        ),
        out_shardings=tuple(NamedSharding(mesh, pspec) for _ in zero_shapes),
    )

    def run(in_maps):
        concat_in = [
            np.concatenate([m[name] for m in in_maps], axis=0) for name in in_names
        ]
        dev_in = [jax.device_put(a, NamedSharding(mesh, pspec)) for a in concat_in]
        out = sharded(*dev_in, *zeros_fn())
        results = []
        for c in range(NCORES):
            results.append(
                {
                    name: np.asarray(out[i]).reshape(NCORES, *out_avals[i].shape)[c]
                    for i, name in enumerate(out_names)
                }
            )
        return results, dev_in

    def bench(dev_in, iters=3):
        import time

        times = []
        for _ in range(iters):
            zs = jax.block_until_ready(zeros_fn())
            t0 = time.perf_counter()
            out = sharded(*dev_in, *zs)
            jax.block_until_ready(out)
            times.append(time.perf_counter() - t0)
        return times

    return run, bench, in_names


def _default_cfg():
    use_f32r = os.environ.get("TRN_F32R", "1") == "1"
    return dict(use_f32r=use_f32r, ileave=4, pack_ew=True)


def _get_runner():
    if "runner" in _CACHE:
        return _CACHE["runner"]
    nc = build_nc(NTILES, **_default_cfg())
    _CACHE["runner"] = _make_runner(nc)
    return _CACHE["runner"]


def bench_exec(iters=3):
    """Steady-state wall time per execution (device-resident inputs)."""
    run, bench, _ = _get_runner()
    dev_in = _CACHE.get("last_dev_in")
    if dev_in is None:
        raise RuntimeError("call kernel() first")
    return bench(dev_in, iters)


def bench_exec_reps(reps=6, iters=8):
    """Estimate true per-execution device time: run the kernel wrapped in an
    on-device For_i(reps) loop and difference against the reps=1 wall time,
    cancelling the fixed axon-RPC/dispatch overhead. Returns ns (int)."""
    import jax

    in_maps = _CACHE.get("last_in_maps")
    if in_maps is None:
        raise RuntimeError("call kernel() first")
    run1, bench1, _ = _get_runner()
    dev_in1 = _CACHE["last_dev_in"]
    ncr = build_nc(NTILES, reps=reps, **_default_cfg())
    runR, benchR, in_namesR = _make_runner(ncr)
    _, dev_inR = runR(in_maps)  # warm-up / compile
    t1 = bench1(dev_in1, iters)
    tR = benchR(dev_inR, iters)
    est = (min(tR) - min(t1)) / (reps - 1)
    return int(max(est, 0.0) * 1e9)


def kernel(context, embed_table, start_embed, Wp, bp, Wi, Wh, bh, Wo, bo, seed):
    global LAST_RESULTS
    run, _, _ = _get_runner()
    in_maps, Gp, bo_np = _prep_host(
        context, embed_table, start_embed, Wp, bp, Wi, Wh, bh, Wo, bo, seed
    )
    results, dev_in = run(in_maps)
    _CACHE["last_dev_in"] = dev_in
    _CACHE["last_in_maps"] = in_maps
    res = type("R", (), {"results": results})()
    LAST_RESULTS = res

    logits = np.empty((B, T, V), np.float32)
    samples = np.empty((B, T), np.int32)
    for c in range(NCORES):
        lc = res.results[c]["louts"]  # (NTILES, 128, 2, T, V)
        # device z == lc + gumb bit-exactly -> samples match device feedback
        zc = lc + in_maps[c]["gumb"]
        sc = np.argmax(zc, axis=-1)  # (NTILES, 128, 2, T)
        sl = slice(c * BLOC, (c + 1) * BLOC)
        logits[sl] = lc.transpose(0, 2, 1, 3, 4).reshape(BLOC, T, V)
        samples[sl] = sc.transpose(0, 2, 1, 3).reshape(BLOC, T).astype(np.int32)
    logits += bo_np[None, None, :]
    return logits, samples


# revision 43
# speedup vs baseline: 27.2367x; 1.5088x over previous
"""Trainium2 Bass kernel for nn_BeliefDecoder (LSTM decoder with categorical
sampling), data-parallel over 8 NeuronCores.

Contract: kernel(**inputs) takes FULL unsharded inputs (as produced by
setup_inputs()) and returns the FULL output tuple
(logits (B, 15, 26) f32, samples (B, 15) int32).

Strategy
--------
- Pure data parallel: batch 65536 -> 8 cores x 8192 rows; weights replicated.
- Sampling must be bit-identical to jax.random.categorical: Gumbel noise is
  precomputed on host (CPU jax, threefry -> bit-exact) and shipped to the
  device. jax.random.categorical(k, logits) == argmax(gumbel(k, shape) +
  logits).
- On device, everything runs in a transposed layout [feature/vocab partitions,
  batch free]:
    h0 = tanh(ctx @ Wp + bp)                      (PE + ACT)
    per step: gates = Wh.h + EW'[tok] + xc + b    (PE accumulate + DVE add)
              i,f,g,o nonlinearities              (ACT)
              c,h update                          (DVE)
              logits = Wo.h                       (PE)
              z = logits + G[t]                   (DVE)
              m = max over vocab partitions       (GPSIMD partition_all_reduce)
              onehot = (z == m)                   (DVE is_equal)
  where xc = ctx @ Wi[:256] is precomputed once per batch-tile (context is
  time-invariant) and EW' = embed_table @ Wi[256:] + bh folds the embedding
  lookup into a tiny 26-row matmul against the onehot (the sampled token feeds
  back without ever materialising embeddings).
- The samples themselves are recovered on the host from the logits the kernel
  already outputs: argmax(logits + G) in f32 is bit-identical to the device's
  (z == m) selection.
- Matmuls run in true fp32 (4-pass H/L) so the sampled trajectory tracks the
  f32 reference closely enough that argmax flips are rare.
"""

import os
import numpy as np

H = 256          # hidden
T = 15           # decode steps (num_components)
V = 26           # vocab
E = 64           # embed dim
B = 65536        # batch
NCORES = 8
BT = 256         # batch tile (moving free dim per matmul)
BLOC = B // NCORES
NTILES = BLOC // BT  # 32

_CACHE = {}
LAST_RESULTS = None


def build_nc(n_tiles, use_f32r=False, ileave=2, prod_gp=False,
             pg_bufs=3, kbufs=3, pack_ew=False, reps=1, off_gp=False,
             bt=None):
    """Build the Bass/Tile program for one core handling n_tiles*BT rows."""
    import concourse.bass as bass
    import concourse.tile as tile
    import concourse.mybir as mybir
    from concourse import bacc

    f32 = mybir.dt.float32
    wdt = mybir.dt.float32r if use_f32r else f32
    Sig = mybir.ActivationFunctionType.Sigmoid
    Tanh = mybir.ActivationFunctionType.Tanh
    ADD = mybir.AluOpType.add
    ISEQ = mybir.AluOpType.is_equal
    bt = BT if bt is None else bt
    bloc = n_tiles * bt
    nc2 = bt // 128  # batch sub-chunks of 128 for batch-major logits/sampling
    gpq = 512 // bt  # gates per PSUM group: BT=256 -> 2 (i,g)/(f,o); BT=512 -> 1

    nc = bacc.Bacc("TRN2", target_bir_lowering=False, debug=False)

    ctxT = nc.dram_tensor("ctxT", (H, bloc), wdt, kind="ExternalInput")
    gumb = nc.dram_tensor("gumb", (n_tiles, 128, nc2, T, V), f32, kind="ExternalInput")
    wp_d = nc.dram_tensor("wp", (H, H), wdt, kind="ExternalInput")
    wi_d = nc.dram_tensor("wi1", (H, 4 * H), wdt, kind="ExternalInput")
    wh_d = nc.dram_tensor("wh", (H, 4 * H), wdt, kind="ExternalInput")
    wo_d = nc.dram_tensor("wo", (H, V), wdt, kind="ExternalInput")
    ew_d = nc.dram_tensor("ew", (V, 4 * H), wdt, kind="ExternalInput")
    cst_d = nc.dram_tensor("consts", (128, 10), f32, kind="ExternalInput")
    id_d = nc.dram_tensor("ident", (128, 128), f32, kind="ExternalInput")
    louts = nc.dram_tensor(
        "louts", (n_tiles, 128, nc2, T, V), f32, kind="ExternalOutput"
    )

    def mm(out, lhsT, rhs, family, **kw):
        nc.tensor.matmul(out, lhsT, rhs, **kw)

    # gate processing groups: c-update needs i,g early; h-update needs f,o late.
    # chunk c of 4H belongs to gate c//2 (order i,f,g,o).
    if gpq == 2:
        GROUPS = [[0, 2], [1, 3]]      # (i,g) then (f,o)
    else:
        GROUPS = [[0], [2], [1], [3]]  # i, g, f, o

    with tile.TileContext(nc) as tc:
        with (
            tc.tile_pool(name="weights", bufs=1) as wpool,
            tc.tile_pool(name="state", bufs=ileave + 1) as spool,
            tc.tile_pool(name="work", bufs=kbufs) as kpool,
            tc.tile_pool(name="oh", bufs=2 * ileave) as ohpool,
            tc.tile_pool(name="stage", bufs=ileave + 1) as stpool,
            tc.tile_pool(name="pgates", bufs=pg_bufs, space="PSUM") as pg_pool,
            tc.tile_pool(name="plog", bufs=1, space="PSUM") as pl_pool,
            tc.tile_pool(name="pbc", bufs=1, space="PSUM") as pbc_pool,
        ):
            # ---- load weights (once) ----
            wp_sb = wpool.tile([128, 2, 2, 128], wdt, tag="wp")
            nc.sync.dma_start(
                wp_sb[:], wp_d.rearrange("(k p) (m c) -> p k m c", p=128, c=128)
            )
            wi_sb = wpool.tile([128, 2, 8, 128], wdt, tag="wi")
            nc.sync.dma_start(
                wi_sb[:], wi_d.rearrange("(k p) (m c) -> p k m c", p=128, c=128)
            )
            wh_sb = wpool.tile([128, 2, 8, 128], wdt, tag="wh")
            nc.sync.dma_start(
                wh_sb[:], wh_d.rearrange("(k p) (m c) -> p k m c", p=128, c=128)
            )
            wo_sb = wpool.tile([128, 2, V], wdt, tag="wo")
            nc.sync.dma_start(wo_sb[:], wo_d.rearrange("(k p) v -> p k v", p=128))
            if pack_ew:
                # EW replicated at partition offsets 0/32/64/96 for 4-way
                # row-group packed matmuls
                ew_sb = wpool.tile([128, 8, 128], wdt, tag="ew")
                for r in range(4):
                    nc.sync.dma_start(
                        ew_sb[32 * r : 32 * r + V, :, :],
                        ew_d.rearrange("v (m c) -> v m c", c=128),
                    )
            else:
                ew_sb = wpool.tile([V, 8, 128], wdt, tag="ew")
                nc.sync.dma_start(ew_sb[:], ew_d.rearrange("v (m c) -> v m c", c=128))
            cst = wpool.tile([128, 10], f32, tag="cst")
            nc.sync.dma_start(cst[:], cst_d[:, :])
            id_sb = wpool.tile([128, 128], f32, tag="ident")
            nc.sync.dma_start(id_sb[:], id_d[:, :])

            state = {}  # per-tile persistent tiles
            kpool_last_tct = [None]
            prev_oh = {}

            def setup(j):
                ctx_sb = spool.tile([128, 2, bt], wdt, tag="ctx")
                nc.sync.dma_start(
                    ctx_sb[:],
                    ctxT.rearrange("(k p) b -> p k b", p=128)[
                        :, :, j * bt : (j + 1) * bt
                    ],
                )
                g_sb = stpool.tile([128, nc2, T, V], f32, tag="gum")
                nc.sync.dma_start(g_sb[:], gumb[j])

                # h0 = tanh(ctx @ Wp + bp)
                hT = spool.tile([128, 2, bt], wdt, tag="h")
                cT = spool.tile([128, 2, bt], f32, tag="c")
                nc.vector.memset(cT[:], 0.0)
                p_wp = pg_pool.tile([128, 2 * gpq, bt], f32, tag="pg")
                for m in range(2):
                    for k in range(2):
                        mm(
                            p_wp[:, m, :], wp_sb[:, k, m, :], ctx_sb[:, k, :],
                            "wp", start=(k == 0), stop=(k == 1),
                        )
                for m in range(2):
                    nc.scalar.activation(
                        hT[:, m, :], p_wp[:, m, :], Tanh, bias=cst[:, 8 + m : 9 + m]
                    )

                lstage = stpool.tile([128, nc2, T, V], f32, tag="lst")
                state[j] = (hT, cT, ctx_sb, g_sb, lstage)

            def step(j, t):
                hT, cT, ctx_sb, g_sb, lstage = state[j]
                # gates, PSUM groups of 2 banks each:
                #   gates = Wh.h + Wi[:256].ctx (+ EW'[tok_prev]); ACT reads PSUM
                acts = kpool.tile([128, 4, 2, bt], f32, tag="acts")
                t1 = kpool.tile([128, 2, bt], f32, tag="t1")
                t2 = kpool.tile([128, 2, bt], f32, tag="t2")
                nrg = 2 * gpq  # row groups used for packed EW matmuls
                for gi, gates in enumerate(GROUPS):
                    p_h = pg_pool.tile([128, 2 * gpq, bt], f32, tag="pg")
                    for pg_i, gate in enumerate(gates):
                        for c2 in range(2):
                            pos = pg_i * 2 + c2
                            m = gate * 2 + c2
                            for k in range(2):
                                mm(
                                    p_h[:, pos, :], wh_sb[:, k, m, :], hT[:, k, :],
                                    "wh", start=(k == 0), stop=False,
                                )
                            for k in range(2):
                                mm(
                                    p_h[:, pos, :], wi_sb[:, k, m, :],
                                    ctx_sb[:, k, :],
                                    "xc", start=False, stop=(t == 0 and k == 1),
                                )
                            if t > 0:
                                if pack_ew:
                                    r = pos  # row group = position within group
                                    nc.tensor.matmul(
                                        p_h[:, pos, :],
                                        ew_sb[32 * r : 32 * r + V, m, :],
                                        prev_oh[j][32 * r : 32 * r + V, :],
                                        start=False, stop=True,
                                        tile_position=(32 * r, 0),
                                    )
                                else:
                                    mm(
                                        p_h[:, pos, :], ew_sb[:, m, :],
                                        prev_oh[j][0:V, :],
                                        "ew", start=False, stop=True,
                                    )
                    # nonlinearities straight from PSUM; bias bh(+sE) only at
                    # t=0 (EW' carries bh for t>0)
                    for pg_i, gate in enumerate(gates):
                        func = Tanh if gate == 2 else Sig
                        if t == 0:
                            for c2 in range(2):
                                m = gate * 2 + c2
                                nc.scalar.activation(
                                    acts[:, gate, c2, :],
                                    p_h[:, pg_i * 2 + c2, :], func,
                                    bias=cst[:, m : m + 1],
                                )
                        else:
                            nc.scalar.activation(
                                acts[:, gate, :, :],
                                p_h[:, pg_i * 2 : pg_i * 2 + 2, :], func,
                            )
                    done = set(g for gg in GROUPS[: gi + 1] for g in gg)
                    just = set(gates)
                    mule = nc.gpsimd if prod_gp else nc.vector
                    if 2 in just and 0 in done:
                        # t2 = sig(i)*tanh(g) (off critical path)
                        (nc.gpsimd if off_gp else mule).tensor_mul(
                            t2[:], acts[:, 0, :, :], acts[:, 2, :, :]
                        )
                    if 1 in just:
                        # c = sig(f)*c + t2
                        mule.tensor_mul(t1[:], acts[:, 1, :, :], cT[:])
                        nc.vector.tensor_add(cT[:], t1[:], t2[:])
                        tct = kpool.tile([128, 2, bt], f32, tag="tct")
                        nc.scalar.activation(tct[:], cT[:], Tanh)
                        kpool_last_tct[0] = tct
                    if 3 in just:
                        # h = sig(o)*tanh(c)
                        tct = kpool_last_tct[0]
                        mule.tensor_mul(hT[:], acts[:, 3, :, :], tct[:])

                # logits in BATCH-major: out[batch 128, 26] via hT-as-stationary
                p_lb = pl_pool.tile([128, nc2, V], f32, tag="plb")
                for c in range(nc2):
                    for k in range(2):
                        mm(
                            p_lb[:, c, :],
                            hT[:, k, c * 128 : (c + 1) * 128],
                            wo_sb[:, k, :],
                            "wo", start=(k == 0), stop=(k == 1),
                        )
                nc.scalar.copy(lstage[:, :, t, :], p_lb[:])

                if t < T - 1:
                    # z = logits + gumbel'; argmax along the free (vocab) dim;
                    # onehot back to vocab-major via PE transpose for feedback.
                    zt = kpool.tile([128, nc2, V], f32, tag="zt")
                    nc.vector.tensor_add(zt[:], p_lb[:], g_sb[:, :, t, :])
                    mt = kpool.tile([128, nc2], f32, tag="mt")
                    nc.vector.tensor_reduce(
                        mt[:], zt[:], axis=mybir.AxisListType.X,
                        op=mybir.AluOpType.max,
                    )
                    eqt = kpool.tile([128, nc2, V], f32, tag="eqt")
                    for c in range(nc2):
                        nc.vector.tensor_scalar(
                            eqt[:, c, :], zt[:, c, :], mt[:, c : c + 1], None,
                            op0=ISEQ,
                        )
                    p_oh = pbc_pool.tile([V, nc2, 128], f32, tag="poh")
                    for c in range(nc2):
                        nc.tensor.transpose(p_oh[:, c, :], eqt[:, c, :], id_sb[:])
                    if pack_ew:
                        oh = ohpool.tile([32 * (nrg - 1) + V, bt], wdt, tag="oh")
                        nc.scalar.copy(oh[0:V, :], p_oh[:])
                        for r in range(1, nrg):
                            (nc.gpsimd if off_gp else nc.vector).tensor_copy(
                                oh[32 * r : 32 * r + V, :], oh[0:V, :]
                            )
                    else:
                        oh = ohpool.tile([V, bt], wdt, tag="oh")
                        nc.scalar.copy(oh[:], p_oh[:])
                    prev_oh[j] = oh

            def finish(j):
                lstage = state[j][4]
                nc.sync.dma_start(louts[j], lstage[:])
                del state[j]
                prev_oh.pop(j, None)

            def whole_pass():
                for jj in range(0, n_tiles, ileave):
                    grp = list(range(jj, min(jj + ileave, n_tiles)))
                    for j in grp:
                        setup(j)
                    for t in range(T):
                        for j in grp:
                            step(j, t)
                    for j in grp:
                        finish(j)

            if reps == 1:
                whole_pass()
            else:
                # benchmark-only: repeat the whole computation on-device so
                # the per-execution time is resolvable above host RPC noise
                with tc.For_i(0, reps, 1):
                    whole_pass()

    nc.compile()
    return nc


def _prep_host(context, embed_table, start_embed, Wp, bp, Wi, Wh, bh, Wo, bo, seed):
    """Host-side preprocessing: gumbel noise, weight repacks, per-core shards."""
    import jax
    import jax.numpy as jnp

    cpu = jax.devices("cpu")[0]
    with jax.default_device(cpu):
        keys = jax.random.split(jax.random.key(int(seed)), T)
        # per-key gumbel calls: bit-exact with jax.random.categorical's
        # internal noise (NOTE: vmap over keys yields different bits!)
        gfn = jax.jit(lambda k: jax.random.gumbel(k, (B, V), jnp.float32))
        G = np.stack([np.asarray(gfn(keys[t])) for t in range(T)])  # (T, B, V)
    bo = np.asarray(bo, np.float32)
    Gp = (G + bo[None, None, :]).astype(np.float32)  # z = logits + (gumbel + bo)

    Wi = np.asarray(Wi)
    bh = np.asarray(bh)
    ew = (
        np.asarray(embed_table).astype(np.float64) @ Wi[H:].astype(np.float64)
        + bh.astype(np.float64)
    ).astype(np.float32)  # (V, 4H)
    bias0 = (
        np.asarray(start_embed).astype(np.float64) @ Wi[H:].astype(np.float64)
        + bh.astype(np.float64)
    ).astype(np.float32)  # (4H,)
    consts = np.zeros((128, 10), np.float32)
    for ch in range(8):
        consts[:, ch] = bias0[ch * 128 : (ch + 1) * 128]
    bp = np.asarray(bp, np.float32)
    consts[:, 8] = bp[:128]
    consts[:, 9] = bp[128:]

    context = np.asarray(context, np.float32)
    in_maps = []
    for c in range(NCORES):
        sl = slice(c * BLOC, (c + 1) * BLOC)
        ctxT = np.ascontiguousarray(context[sl].T)  # (H, BLOC)
        # (T, BLOC, V) -> (NTILES, 128, BT//128, T, V); b = j*BT + c*128 + p
        gc = np.ascontiguousarray(
            Gp[:, sl, :]
            .reshape(T, NTILES, BT // 128, 128, V)
            .transpose(1, 3, 2, 0, 4)
        )
        in_maps.append(
            dict(
                ctxT=ctxT,
                gumb=gc,
                wp=np.asarray(Wp, np.float32),
                wi1=Wi[:H].astype(np.float32),
                wh=np.asarray(Wh, np.float32),
                wo=np.asarray(Wo, np.float32),
                ew=ew,
                consts=consts,
                ident=np.eye(128, dtype=np.float32),
            )
        )
    return in_maps, Gp, bo


def _make_runner(nc):
    """Build a sharded-PJRT executable for a built Bass program, modeled on
    concourse.bass2jax.run_bass_via_pjrt's multi-core path, plus a bench()
    that times steady-state execution with device-resident inputs."""
    import jax
    import jax.numpy as jnp
    from jax.sharding import Mesh, PartitionSpec, NamedSharding
    from jax.experimental.shard_map import shard_map
    import concourse.mybir as mybir
    from concourse import bass2jax

    bass2jax.install_neuronx_cc_hook()

    partition_name = (
        nc.partition_id_tensor.name if nc.partition_id_tensor else None
    )
    in_names, out_names, out_avals, zero_shapes = [], [], [], []
    for alloc in nc.m.functions[0].allocations:
        if not isinstance(alloc, mybir.MemoryLocationSet):
            continue
        name = alloc.memorylocations[0].name
        if alloc.kind == "ExternalInput":
            if name != partition_name:
                in_names.append(name)
        elif alloc.kind == "ExternalOutput":
            shape = tuple(alloc.tensor_shape)
            dtype = mybir.dt.np(alloc.dtype)
            out_names.append(name)
            out_avals.append(jax.core.ShapedArray(shape, dtype))
            zero_shapes.append((shape, dtype))
    n_params = len(in_names)
    n_outs = len(out_names)
    all_names = tuple(
        in_names + out_names + ([partition_name] if partition_name else [])
    )

    def _body(*args):
        operands = list(args)
        if partition_name is not None:
            operands.append(bass2jax.partition_id_tensor())
        outs = bass2jax._bass_exec_p.bind(
            *operands,
            out_avals=tuple(out_avals),
            in_names=all_names,
            out_names=tuple(out_names),
            lowering_input_output_aliases=(),
            sim_require_finite=True,
            sim_require_nnan=True,
            nc=nc,
        )
        return tuple(outs)

    devices = jax.devices()[:NCORES]
    mesh = Mesh(np.asarray(devices), ("core",))
    pspec = PartitionSpec("core")
    sharded = jax.jit(
        shard_map(
            _body,
            mesh=mesh,
            in_specs=(pspec,) * (n_params + n_outs),
            out_specs=(pspec,) * n_outs,
            check_rep=False,
        ),
        donate_argnums=tuple(range(n_params, n_params + n_outs)),
        keep_unused=True,
    )
    zeros_fn = jax.jit(
        lambda: tuple(
            jnp.zeros((NCORES * s[0], *s[1:]), d) for s, d in zero_shapes
        ),
        out_shardings=tuple(NamedSharding(mesh, pspec) for _ in zero_shapes),
    )

    def run(in_maps):
        concat_in = [
            np.concatenate([m[name] for m in in_maps], axis=0) for name in in_names
        ]
        dev_in = [jax.device_put(a, NamedSharding(mesh, pspec)) for a in concat_in]
        out = sharded(*dev_in, *zeros_fn())
        results = []
        for c in range(NCORES):
            results.append(
                {
                    name: np.asarray(out[i]).reshape(NCORES, *out_avals[i].shape)[c]
                    for i, name in enumerate(out_names)
                }
            )
        return results, dev_in

    def bench(dev_in, iters=3):
        import time

        times = []
        for _ in range(iters):
            zs = jax.block_until_ready(zeros_fn())
            t0 = time.perf_counter()
            out = sharded(*dev_in, *zs)
            jax.block_until_ready(out)
            times.append(time.perf_counter() - t0)
        return times

    return run, bench, in_names


def _apply_bt_env():
    """Honor TRN_BT={256,512} (default 256) by adjusting module tiling."""
    global BT, NTILES
    bt = int(os.environ.get("TRN_BT", "256"))
    if bt != BT:
        BT = bt
        NTILES = BLOC // BT


def _default_cfg():
    _apply_bt_env()
    use_f32r = os.environ.get("TRN_F32R", "1") == "1"
    if BT == 512:
        return dict(use_f32r=use_f32r, ileave=2, kbufs=2, pack_ew=True, bt=512)
    return dict(use_f32r=use_f32r, ileave=4, pack_ew=True)


def _get_runner():
    _apply_bt_env()
    if "runner" in _CACHE:
        return _CACHE["runner"]
    nc = build_nc(NTILES, **_default_cfg())
    _CACHE["runner"] = _make_runner(nc)
    return _CACHE["runner"]


def bench_exec(iters=3):
    """Steady-state wall time per execution (device-resident inputs)."""
    run, bench, _ = _get_runner()
    dev_in = _CACHE.get("last_dev_in")
    if dev_in is None:
        raise RuntimeError("call kernel() first")
    return bench(dev_in, iters)


def bench_exec_reps(reps=6, iters=8):
    """Estimate true per-execution device time: run the kernel wrapped in an
    on-device For_i(reps) loop and difference against the reps=1 wall time,
    cancelling the fixed axon-RPC/dispatch overhead. Returns ns (int)."""
    import jax

    in_maps = _CACHE.get("last_in_maps")
    if in_maps is None:
        raise RuntimeError("call kernel() first")
    run1, bench1, _ = _get_runner()
    dev_in1 = _CACHE["last_dev_in"]
    ncr = build_nc(NTILES, reps=reps, **_default_cfg())
    runR, benchR, in_namesR = _make_runner(ncr)
    _, dev_inR = runR(in_maps)  # warm-up / compile
    t1 = bench1(dev_in1, iters)
    tR = benchR(dev_inR, iters)
    est = (min(tR) - min(t1)) / (reps - 1)
    return int(max(est, 0.0) * 1e9)


def kernel(context, embed_table, start_embed, Wp, bp, Wi, Wh, bh, Wo, bo, seed):
    global LAST_RESULTS
    run, _, _ = _get_runner()
    in_maps, Gp, bo_np = _prep_host(
        context, embed_table, start_embed, Wp, bp, Wi, Wh, bh, Wo, bo, seed
    )
    results, dev_in = run(in_maps)
    _CACHE["last_dev_in"] = dev_in
    _CACHE["last_in_maps"] = in_maps
    res = type("R", (), {"results": results})()
    LAST_RESULTS = res

    logits = np.empty((B, T, V), np.float32)
    samples = np.empty((B, T), np.int32)
    for c in range(NCORES):
        lc = res.results[c]["louts"]  # (NTILES, 128, 2, T, V)
        # device z == lc + gumb bit-exactly -> samples match device feedback
        zc = lc + in_maps[c]["gumb"]
        sc = np.argmax(zc, axis=-1)  # (NTILES, 128, 2, T)
        sl = slice(c * BLOC, (c + 1) * BLOC)
        logits[sl] = lc.transpose(0, 2, 1, 3, 4).reshape(BLOC, T, V)
        samples[sl] = sc.transpose(0, 2, 1, 3).reshape(BLOC, T).astype(np.int32)
    logits += bo_np[None, None, :]
    return logits, samples


# revision 45
# speedup vs baseline: 28.9731x; 1.0637x over previous
"""Trainium2 Bass kernel for nn_BeliefDecoder (LSTM decoder with categorical
sampling), data-parallel over 8 NeuronCores.

Contract: kernel(**inputs) takes FULL unsharded inputs (as produced by
setup_inputs()) and returns the FULL output tuple
(logits (B, 15, 26) f32, samples (B, 15) int32).

Strategy
--------
- Pure data parallel: batch 65536 -> 8 cores x 8192 rows; weights replicated.
- Sampling must be bit-identical to jax.random.categorical: Gumbel noise is
  precomputed on host (CPU jax, threefry -> bit-exact, one gumbel() call per
  split key) and shipped to the device.
  jax.random.categorical(k, logits) == argmax(gumbel(k, shape) + logits).
- Per core, the batch is processed as tiles of BT rows, a few tiles
  interleaved in flight so PE never idles through the sampling chain.
  Gates/h/c live in a transposed layout [feature partitions, batch free]:
    h0 = tanh(ctx @ Wp + bp)                        (PE + ACT)
    per step, per gate-group (PSUM 2-bank groups):
      gates = Wh.h + Wi[:256].ctx + EW'[tok_prev]   (PE fp32r accumulate;
              the ctx term is recomputed each step - at 1 cyc/row that is
              cheaper than draining a precomputed xc through the DVE;
              EW' matmuls are 26-row, packed 2-up via tile_position)
      i,f,g,o nonlinearities straight from PSUM      (ACT)
      c,h update                                     (DVE)
    logits = h @ Wo in BATCH-major [batch parts, 26] (PE, hT as stationary)
    z = logits + G[t]; m = rowmax; onehot = (z == m) (DVE, free-dim reduce)
    onehot -> vocab-major via PE transpose, feeds next step's EW' matmul.
  EW' = embed_table @ Wi[256:] + bh folds the embedding lookup into a tiny
  26-row matmul against the onehot - the sampled token feeds back without
  ever materialising embeddings; start_embed & biases fold into the t=0
  ACT bias.
- The samples are recovered on the host from the logits the kernel already
  outputs: argmax(logits + G) in f32 is bit-identical to the device's
  (z == m) selection.
- Matmuls run in float32r (FP22 inputs, 1 cyc/row - 4x fp32). Measured
  effect vs the f32 CPU reference: ~30 of 983k sampled tokens flip
  (samples rel err ~4e-3, logits ~2.6e-4). Set TRN_F32R=0 for true fp32
  (zero flips, ~2.5x slower).
"""

import os
import numpy as np

H = 256          # hidden
T = 15           # decode steps (num_components)
V = 26           # vocab
E = 64           # embed dim
B = 65536        # batch
NCORES = 8
BT = 256         # batch tile (moving free dim per matmul)
BLOC = B // NCORES
NTILES = BLOC // BT  # 32

_CACHE = {}
LAST_RESULTS = None


def build_nc(n_tiles, use_f32r=False, ileave=2, prod_gp=False,
             pg_bufs=3, kbufs=3, pack_ew=False, reps=1, off_gp=False,
             bt=None):
    """Build the Bass/Tile program for one core handling n_tiles*BT rows."""
    import concourse.bass as bass
    import concourse.tile as tile
    import concourse.mybir as mybir
    from concourse import bacc

    f32 = mybir.dt.float32
    wdt = mybir.dt.float32r if use_f32r else f32
    Sig = mybir.ActivationFunctionType.Sigmoid
    Tanh = mybir.ActivationFunctionType.Tanh
    ADD = mybir.AluOpType.add
    ISEQ = mybir.AluOpType.is_equal
    bt = BT if bt is None else bt
    bloc = n_tiles * bt
    nc2 = bt // 128  # batch sub-chunks of 128 for batch-major logits/sampling
    gpq = 512 // bt  # gates per PSUM group: BT=256 -> 2 (i,g)/(f,o); BT=512 -> 1

    nc = bacc.Bacc("TRN2", target_bir_lowering=False, debug=False)

    ctxT = nc.dram_tensor("ctxT", (H, bloc), wdt, kind="ExternalInput")
    gumb = nc.dram_tensor("gumb", (n_tiles, 128, nc2, T, V), f32, kind="ExternalInput")
    wp_d = nc.dram_tensor("wp", (H, H), wdt, kind="ExternalInput")
    wi_d = nc.dram_tensor("wi1", (H, 4 * H), wdt, kind="ExternalInput")
    wh_d = nc.dram_tensor("wh", (H, 4 * H), wdt, kind="ExternalInput")
    wo_d = nc.dram_tensor("wo", (H, V), wdt, kind="ExternalInput")
    ew_d = nc.dram_tensor("ew", (V, 4 * H), wdt, kind="ExternalInput")
    cst_d = nc.dram_tensor("consts", (128, 10), f32, kind="ExternalInput")
    id_d = nc.dram_tensor("ident", (128, 128), f32, kind="ExternalInput")
    louts = nc.dram_tensor(
        "louts", (n_tiles, 128, nc2, T, V), f32, kind="ExternalOutput"
    )

    def mm(out, lhsT, rhs, family, **kw):
        nc.tensor.matmul(out, lhsT, rhs, **kw)

    # gate processing groups: c-update needs i,g early; h-update needs f,o late.
    # chunk c of 4H belongs to gate c//2 (order i,f,g,o).
    if gpq == 2:
        GROUPS = [[0, 2], [1, 3]]      # (i,g) then (f,o)
    else:
        GROUPS = [[0], [2], [1], [3]]  # i, g, f, o

    with tile.TileContext(nc) as tc:
        with (
            tc.tile_pool(name="weights", bufs=1) as wpool,
            tc.tile_pool(name="state", bufs=ileave + 1) as spool,
            tc.tile_pool(name="work", bufs=kbufs) as kpool,
            tc.tile_pool(name="oh", bufs=2 * ileave) as ohpool,
            tc.tile_pool(name="stage", bufs=ileave + 1) as stpool,
            tc.tile_pool(name="pgates", bufs=pg_bufs, space="PSUM") as pg_pool,
            tc.tile_pool(name="plog", bufs=1, space="PSUM") as pl_pool,
            tc.tile_pool(name="pbc", bufs=1, space="PSUM") as pbc_pool,
        ):
            # ---- load weights (once) ----
            wp_sb = wpool.tile([128, 2, 2, 128], wdt, tag="wp")
            nc.sync.dma_start(
                wp_sb[:], wp_d.rearrange("(k p) (m c) -> p k m c", p=128, c=128)
            )
            wi_sb = wpool.tile([128, 2, 8, 128], wdt, tag="wi")
            nc.sync.dma_start(
                wi_sb[:], wi_d.rearrange("(k p) (m c) -> p k m c", p=128, c=128)
            )
            wh_sb = wpool.tile([128, 2, 8, 128], wdt, tag="wh")
            nc.sync.dma_start(
                wh_sb[:], wh_d.rearrange("(k p) (m c) -> p k m c", p=128, c=128)
            )
            wo_sb = wpool.tile([128, 2, V], wdt, tag="wo")
            nc.sync.dma_start(wo_sb[:], wo_d.rearrange("(k p) v -> p k v", p=128))
            if pack_ew:
                # EW replicated at partition offsets 0/32/64/96 for 4-way
                # row-group packed matmuls
                ew_sb = wpool.tile([128, 8, 128], wdt, tag="ew")
                for r in range(4):
                    nc.sync.dma_start(
                        ew_sb[32 * r : 32 * r + V, :, :],
                        ew_d.rearrange("v (m c) -> v m c", c=128),
                    )
            else:
                ew_sb = wpool.tile([V, 8, 128], wdt, tag="ew")
                nc.sync.dma_start(ew_sb[:], ew_d.rearrange("v (m c) -> v m c", c=128))
            cst = wpool.tile([128, 10], f32, tag="cst")
            nc.sync.dma_start(cst[:], cst_d[:, :])
            id_sb = wpool.tile([128, 128], f32, tag="ident")
            nc.sync.dma_start(id_sb[:], id_d[:, :])

            state = {}  # per-tile persistent tiles
            kpool_last_tct = [None]
            prev_oh = {}

            def setup(j):
                ctx_sb = spool.tile([128, 2, bt], wdt, tag="ctx")
                nc.sync.dma_start(
                    ctx_sb[:],
                    ctxT.rearrange("(k p) b -> p k b", p=128)[
                        :, :, j * bt : (j + 1) * bt
                    ],
                )
                g_sb = stpool.tile([128, nc2, T, V], f32, tag="gum")
                nc.sync.dma_start(g_sb[:], gumb[j])

                # h0 = tanh(ctx @ Wp + bp)
                hT = spool.tile([128, 2, bt], wdt, tag="h")
                cT = spool.tile([128, 2, bt], f32, tag="c")
                nc.vector.memset(cT[:], 0.0)
                p_wp = pg_pool.tile([128, 2 * gpq, bt], f32, tag="pg")
                for m in range(2):
                    for k in range(2):
                        mm(
                            p_wp[:, m, :], wp_sb[:, k, m, :], ctx_sb[:, k, :],
                            "wp", start=(k == 0), stop=(k == 1),
                        )
                for m in range(2):
                    nc.scalar.activation(
                        hT[:, m, :], p_wp[:, m, :], Tanh, bias=cst[:, 8 + m : 9 + m]
                    )

                lstage = stpool.tile([128, nc2, T, V], f32, tag="lst")
                state[j] = (hT, cT, ctx_sb, g_sb, lstage)

            def step(j, t):
                hT, cT, ctx_sb, g_sb, lstage = state[j]
                # gates, PSUM groups of 2 banks each:
                #   gates = Wh.h + Wi[:256].ctx (+ EW'[tok_prev]); ACT reads PSUM
                acts = kpool.tile([128, 4, 2, bt], f32, tag="acts")
                t1 = kpool.tile([128, 2, bt], f32, tag="t1")
                t2 = kpool.tile([128, 2, bt], f32, tag="t2")
                nrg = 2 * gpq  # row groups used for packed EW matmuls
                for gi, gates in enumerate(GROUPS):
                    p_h = pg_pool.tile([128, 2 * gpq, bt], f32, tag="pg")
                    for pg_i, gate in enumerate(gates):
                        for c2 in range(2):
                            pos = pg_i * 2 + c2
                            m = gate * 2 + c2
                            for k in range(2):
                                mm(
                                    p_h[:, pos, :], wh_sb[:, k, m, :], hT[:, k, :],
                                    "wh", start=(k == 0), stop=False,
                                )
                            for k in range(2):
                                mm(
                                    p_h[:, pos, :], wi_sb[:, k, m, :],
                                    ctx_sb[:, k, :],
                                    "xc", start=False, stop=(t == 0 and k == 1),
                                )
                            if t > 0:
                                if pack_ew:
                                    r = pos  # row group = position within group
                                    nc.tensor.matmul(
                                        p_h[:, pos, :],
                                        ew_sb[32 * r : 32 * r + V, m, :],
                                        prev_oh[j][32 * r : 32 * r + V, :],
                                        start=False, stop=True,
                                        tile_position=(32 * r, 0),
                                    )
                                else:
                                    mm(
                                        p_h[:, pos, :], ew_sb[:, m, :],
                                        prev_oh[j][0:V, :],
                                        "ew", start=False, stop=True,
                                    )
                    # nonlinearities straight from PSUM; bias bh(+sE) only at
                    # t=0 (EW' carries bh for t>0)
                    for pg_i, gate in enumerate(gates):
                        func = Tanh if gate == 2 else Sig
                        if t == 0:
                            for c2 in range(2):
                                m = gate * 2 + c2
                                nc.scalar.activation(
                                    acts[:, gate, c2, :],
                                    p_h[:, pg_i * 2 + c2, :], func,
                                    bias=cst[:, m : m + 1],
                                )
                        else:
                            nc.scalar.activation(
                                acts[:, gate, :, :],
                                p_h[:, pg_i * 2 : pg_i * 2 + 2, :], func,
                            )
                    done = set(g for gg in GROUPS[: gi + 1] for g in gg)
                    just = set(gates)
                    mule = nc.gpsimd if prod_gp else nc.vector
                    if 2 in just and 0 in done:
                        # t2 = sig(i)*tanh(g) (off critical path)
                        (nc.gpsimd if off_gp else mule).tensor_mul(
                            t2[:], acts[:, 0, :, :], acts[:, 2, :, :]
                        )
                    if 1 in just:
                        # c = sig(f)*c + t2
                        mule.tensor_mul(t1[:], acts[:, 1, :, :], cT[:])
                        nc.vector.tensor_add(cT[:], t1[:], t2[:])
                        tct = kpool.tile([128, 2, bt], f32, tag="tct")
                        nc.scalar.activation(tct[:], cT[:], Tanh)
                        kpool_last_tct[0] = tct
                    if 3 in just:
                        # h = sig(o)*tanh(c)
                        tct = kpool_last_tct[0]
                        mule.tensor_mul(hT[:], acts[:, 3, :, :], tct[:])

                # logits in BATCH-major: out[batch 128, 26] via hT-as-stationary
                p_lb = pl_pool.tile([128, nc2, V], f32, tag="plb")
                for c in range(nc2):
                    for k in range(2):
                        mm(
                            p_lb[:, c, :],
                            hT[:, k, c * 128 : (c + 1) * 128],
                            wo_sb[:, k, :],
                            "wo", start=(k == 0), stop=(k == 1),
                        )
                nc.scalar.copy(lstage[:, :, t, :], p_lb[:])

                if t < T - 1:
                    # z = logits + gumbel'; argmax along the free (vocab) dim;
                    # onehot back to vocab-major via PE transpose for feedback.
                    zt = kpool.tile([128, nc2, V], f32, tag="zt")
                    nc.vector.tensor_add(zt[:], p_lb[:], g_sb[:, :, t, :])
                    mt = kpool.tile([128, nc2], f32, tag="mt")
                    nc.vector.tensor_reduce(
                        mt[:], zt[:], axis=mybir.AxisListType.X,
                        op=mybir.AluOpType.max,
                    )
                    eqt = kpool.tile([128, nc2, V], f32, tag="eqt")
                    for c in range(nc2):
                        nc.vector.tensor_scalar(
                            eqt[:, c, :], zt[:, c, :], mt[:, c : c + 1], None,
                            op0=ISEQ,
                        )
                    p_oh = pbc_pool.tile([V, nc2, 128], f32, tag="poh")
                    for c in range(nc2):
                        nc.tensor.transpose(p_oh[:, c, :], eqt[:, c, :], id_sb[:])
                    if pack_ew:
                        oh = ohpool.tile([32 * (nrg - 1) + V, bt], wdt, tag="oh")
                        nc.scalar.copy(oh[0:V, :], p_oh[:])
                        for r in range(1, nrg):
                            (nc.gpsimd if off_gp else nc.vector).tensor_copy(
                                oh[32 * r : 32 * r + V, :], oh[0:V, :]
                            )
                    else:
                        oh = ohpool.tile([V, bt], wdt, tag="oh")
                        nc.scalar.copy(oh[:], p_oh[:])
                    prev_oh[j] = oh

            def finish(j):
                lstage = state[j][4]
                nc.sync.dma_start(louts[j], lstage[:])
                del state[j]
                prev_oh.pop(j, None)

            def whole_pass():
                for jj in range(0, n_tiles, ileave):
                    grp = list(range(jj, min(jj + ileave, n_tiles)))
                    for j in grp:
                        setup(j)
                    for t in range(T):
                        for j in grp:
                            step(j, t)
                    for j in grp:
                        finish(j)

            if reps == 1:
                whole_pass()
            else:
                # benchmark-only: repeat the whole computation on-device so
                # the per-execution time is resolvable above host RPC noise
                with tc.For_i(0, reps, 1):
                    whole_pass()

    nc.compile()
    return nc


def _prep_host(context, embed_table, start_embed, Wp, bp, Wi, Wh, bh, Wo, bo, seed):
    """Host-side preprocessing: gumbel noise, weight repacks, per-core shards."""
    import jax
    import jax.numpy as jnp

    cpu = jax.devices("cpu")[0]
    with jax.default_device(cpu):
        keys = jax.random.split(jax.random.key(int(seed)), T)
        # per-key gumbel calls: bit-exact with jax.random.categorical's
        # internal noise (NOTE: vmap over keys yields different bits!)
        gfn = jax.jit(lambda k: jax.random.gumbel(k, (B, V), jnp.float32))
        G = np.stack([np.asarray(gfn(keys[t])) for t in range(T)])  # (T, B, V)
    bo = np.asarray(bo, np.float32)
    Gp = (G + bo[None, None, :]).astype(np.float32)  # z = logits + (gumbel + bo)

    Wi = np.asarray(Wi)
    bh = np.asarray(bh)
    ew = (
        np.asarray(embed_table).astype(np.float64) @ Wi[H:].astype(np.float64)
        + bh.astype(np.float64)
    ).astype(np.float32)  # (V, 4H)
    bias0 = (
        np.asarray(start_embed).astype(np.float64) @ Wi[H:].astype(np.float64)
        + bh.astype(np.float64)
    ).astype(np.float32)  # (4H,)
    consts = np.zeros((128, 10), np.float32)
    for ch in range(8):
        consts[:, ch] = bias0[ch * 128 : (ch + 1) * 128]
    bp = np.asarray(bp, np.float32)
    consts[:, 8] = bp[:128]
    consts[:, 9] = bp[128:]

    context = np.asarray(context, np.float32)
    in_maps = []
    for c in range(NCORES):
        sl = slice(c * BLOC, (c + 1) * BLOC)
        ctxT = np.ascontiguousarray(context[sl].T)  # (H, BLOC)
        # (T, BLOC, V) -> (NTILES, 128, BT//128, T, V); b = j*BT + c*128 + p
        gc = np.ascontiguousarray(
            Gp[:, sl, :]
            .reshape(T, NTILES, BT // 128, 128, V)
            .transpose(1, 3, 2, 0, 4)
        )
        in_maps.append(
            dict(
                ctxT=ctxT,
                gumb=gc,
                wp=np.asarray(Wp, np.float32),
                wi1=Wi[:H].astype(np.float32),
                wh=np.asarray(Wh, np.float32),
                wo=np.asarray(Wo, np.float32),
                ew=ew,
                consts=consts,
                ident=np.eye(128, dtype=np.float32),
            )
        )
    return in_maps, Gp, bo


def _make_runner(nc):
    """Build a sharded-PJRT executable for a built Bass program, modeled on
    concourse.bass2jax.run_bass_via_pjrt's multi-core path, plus a bench()
    that times steady-state execution with device-resident inputs."""
    import jax
    import jax.numpy as jnp
    from jax.sharding import Mesh, PartitionSpec, NamedSharding
    from jax.experimental.shard_map import shard_map
    import concourse.mybir as mybir
    from concourse import bass2jax

    bass2jax.install_neuronx_cc_hook()

    partition_name = (
        nc.partition_id_tensor.name if nc.partition_id_tensor else None
    )
    in_names, out_names, out_avals, zero_shapes = [], [], [], []
    for alloc in nc.m.functions[0].allocations:
        if not isinstance(alloc, mybir.MemoryLocationSet):
            continue
        name = alloc.memorylocations[0].name
        if alloc.kind == "ExternalInput":
            if name != partition_name:
                in_names.append(name)
        elif alloc.kind == "ExternalOutput":
            shape = tuple(alloc.tensor_shape)
            dtype = mybir.dt.np(alloc.dtype)
            out_names.append(name)
            out_avals.append(jax.core.ShapedArray(shape, dtype))
            zero_shapes.append((shape, dtype))
    n_params = len(in_names)
    n_outs = len(out_names)
    all_names = tuple(
        in_names + out_names + ([partition_name] if partition_name else [])
    )

    def _body(*args):
        operands = list(args)
        if partition_name is not None:
            operands.append(bass2jax.partition_id_tensor())
        outs = bass2jax._bass_exec_p.bind(
            *operands,
            out_avals=tuple(out_avals),
            in_names=all_names,
            out_names=tuple(out_names),
            lowering_input_output_aliases=(),
            sim_require_finite=True,
            sim_require_nnan=True,
            nc=nc,
        )
        return tuple(outs)

    devices = jax.devices()[:NCORES]
    mesh = Mesh(np.asarray(devices), ("core",))
    pspec = PartitionSpec("core")
    sharded = jax.jit(
        shard_map(
            _body,
            mesh=mesh,
            in_specs=(pspec,) * (n_params + n_outs),
            out_specs=(pspec,) * n_outs,
            check_rep=False,
        ),
        donate_argnums=tuple(range(n_params, n_params + n_outs)),
        keep_unused=True,
    )
    zeros_fn = jax.jit(
        lambda: tuple(
            jnp.zeros((NCORES * s[0], *s[1:]), d) for s, d in zero_shapes
        ),
        out_shardings=tuple(NamedSharding(mesh, pspec) for _ in zero_shapes),
    )

    def run(in_maps):
        concat_in = [
            np.concatenate([m[name] for m in in_maps], axis=0) for name in in_names
        ]
        dev_in = [jax.device_put(a, NamedSharding(mesh, pspec)) for a in concat_in]
        out = sharded(*dev_in, *zeros_fn())
        results = []
        for c in range(NCORES):
            results.append(
                {
                    name: np.asarray(out[i]).reshape(NCORES, *out_avals[i].shape)[c]
                    for i, name in enumerate(out_names)
                }
            )
        return results, dev_in

    def bench(dev_in, iters=3):
        import time

        times = []
        for _ in range(iters):
            zs = jax.block_until_ready(zeros_fn())
            t0 = time.perf_counter()
            out = sharded(*dev_in, *zs)
            jax.block_until_ready(out)
            times.append(time.perf_counter() - t0)
        return times

    return run, bench, in_names


def _apply_bt_env():
    """Honor TRN_BT={256,512} (default 256) by adjusting module tiling."""
    global BT, NTILES
    bt = int(os.environ.get("TRN_BT", "512"))
    if bt != BT:
        BT = bt
        NTILES = BLOC // BT


def _default_cfg():
    _apply_bt_env()
    use_f32r = os.environ.get("TRN_F32R", "1") == "1"
    if BT == 512:
        return dict(use_f32r=use_f32r, ileave=2, kbufs=2, pack_ew=True, bt=512)
    return dict(use_f32r=use_f32r, ileave=4, pack_ew=True)


def _get_runner():
    _apply_bt_env()
    if "runner" in _CACHE:
        return _CACHE["runner"]
    nc = build_nc(NTILES, **_default_cfg())
    _CACHE["runner"] = _make_runner(nc)
    return _CACHE["runner"]


def bench_exec(iters=3):
    """Steady-state wall time per execution (device-resident inputs)."""
    run, bench, _ = _get_runner()
    dev_in = _CACHE.get("last_dev_in")
    if dev_in is None:
        raise RuntimeError("call kernel() first")
    return bench(dev_in, iters)


def bench_exec_reps(reps=6, iters=8):
    """Estimate true per-execution device time: run the kernel wrapped in an
    on-device For_i(reps) loop and difference against the reps=1 wall time,
    cancelling the fixed axon-RPC/dispatch overhead. Returns ns (int)."""
    import jax

    in_maps = _CACHE.get("last_in_maps")
    if in_maps is None:
        raise RuntimeError("call kernel() first")
    run1, bench1, _ = _get_runner()
    dev_in1 = _CACHE["last_dev_in"]
    ncr = build_nc(NTILES, reps=reps, **_default_cfg())
    runR, benchR, in_namesR = _make_runner(ncr)
    _, dev_inR = runR(in_maps)  # warm-up / compile
    t1 = bench1(dev_in1, iters)
    tR = benchR(dev_inR, iters)
    est = (min(tR) - min(t1)) / (reps - 1)
    return int(max(est, 0.0) * 1e9)


def kernel(context, embed_table, start_embed, Wp, bp, Wi, Wh, bh, Wo, bo, seed):
    global LAST_RESULTS
    run, _, _ = _get_runner()
    in_maps, Gp, bo_np = _prep_host(
        context, embed_table, start_embed, Wp, bp, Wi, Wh, bh, Wo, bo, seed
    )
    results, dev_in = run(in_maps)
    _CACHE["last_dev_in"] = dev_in
    _CACHE["last_in_maps"] = in_maps
    res = type("R", (), {"results": results})()
    LAST_RESULTS = res

    logits = np.empty((B, T, V), np.float32)
    samples = np.empty((B, T), np.int32)
    for c in range(NCORES):
        lc = res.results[c]["louts"]  # (NTILES, 128, 2, T, V)
        # device z == lc + gumb bit-exactly -> samples match device feedback
        zc = lc + in_maps[c]["gumb"]
        sc = np.argmax(zc, axis=-1)  # (NTILES, 128, 2, T)
        sl = slice(c * BLOC, (c + 1) * BLOC)
        logits[sl] = lc.transpose(0, 2, 1, 3, 4).reshape(BLOC, T, V)
        samples[sl] = sc.transpose(0, 2, 1, 3).reshape(BLOC, T).astype(np.int32)
    logits += bo_np[None, None, :]
    return logits, samples
